# revision 5
# baseline (speedup 1.0000x reference)
"""Distributed multi-head attention kernel for 8 TRN2 NeuronCores.

Problem: hidden[2,2048,1024] -> QKV proj (16 heads, hd=64) -> softmax
attention -> out proj. f32 I/O, bf16 tensor-engine compute; fp8e4
probs/V for a DoubleRow ctx contraction.

Sharding: sequence-parallel. Flattened rows [4096, 1024] split into 8
chunks of 512 rows; cores 0-3 own batch 0, cores 4-7 batch 1. Each core
projects K^T and V for its own 512 rows and AllGathers both within its
4-core batch group (K^T split in two bf16 halves, V as fp8), hidden
under the V/Q projections. Q^T is local. Each core then runs full
16-head attention + output projection for its 512 rows; outputs are
disjoint row blocks concatenated on the host.

PE budget: score matmuls are row-tiled - head 2p contracts on array
rows 0-63, head 2p+1 on rows 64-127 (tile_position from the APs'
base partitions), so the two 64-dim contractions run concurrently and
score throughput doubles vs zero-padded 128-dim matmuls. ctx uses
fp8 DoubleRow over two key tiles per matmul. A warmup matmul burst at
t=0 lifts the HAM clock gate before the first projection.

Engine budget: softmax exp is split three ways - ACT does head-even
tiles (plus every 5th head-odd), the rest run as Schraudolph fast-exp
with the int32 tensor_scalar on the DVE and the bitcast fp8 copy on
the otherwise-idle GPSIMD. Denominators stash in f32, hop through
DRAM, and invert with one reciprocal_approx_fast per head pair; the
per-pair flush interleaves with the ongoing score/ctx stream.
"""

import numpy as np
import ml_dtypes

B, S, D, H, HD = 2, 2048, 1024, 16, 64
N_CORES = 8
ROWS = (B * S) // N_CORES          # 512 query rows per core
GROUP = 4                          # cores per batch group
P = 128
KT = D // P                        # 8 contraction tiles over hidden dim
KEYT = S // P                      # 16 key tiles per batch
HA = HD + 1                        # head slot width in v_aug

_CACHE: dict = {}

bf16 = ml_dtypes.bfloat16
f8 = ml_dtypes.float8_e4m3


def _build_graph():
    import concourse.mybir as mybir
    import concourse.tile as tile
    from concourse import bacc
    from contextlib import ExitStack

    dt = mybir.dt
    F32, BF16, F8 = dt.float32, dt.bfloat16, dt.float8e4
    AF = mybir.ActivationFunctionType
    ALU = mybir.AluOpType
    DR = mybir.MatmulPerfMode.DoubleRow

    nc = bacc.Bacc("TRN2", target_bir_lowering=False, debug=False,
                   enable_asserts=False, num_devices=N_CORES)

    hT = nc.dram_tensor("hT", [D, ROWS], BF16, kind="ExternalInput").ap()
    wq = nc.dram_tensor("wq", [D, D], BF16, kind="ExternalInput").ap()
    wk = nc.dram_tensor("wk", [D, D], BF16, kind="ExternalInput").ap()
    wv = nc.dram_tensor("wv", [D, D], BF16, kind="ExternalInput").ap()
    wo = nc.dram_tensor("wo", [D, D], BF16, kind="ExternalInput").ap()
    bvr = nc.dram_tensor("bvr", [1, D], BF16, kind="ExternalInput").ap()
    bor = nc.dram_tensor("bor", [1, D], BF16, kind="ExternalInput").ap()
    bqk = nc.dram_tensor("bqk", [P, 2 * KT], F32, kind="ExternalInput").ap()
    sel = nc.dram_tensor("sel", [2, 2 * HD], BF16, kind="ExternalInput").ap()
    out = nc.dram_tensor("out", [ROWS, D], F32, kind="ExternalOutput").ap()

    with tile.TileContext(nc) as tc, ExitStack() as top:
        dram = top.enter_context(tc.tile_pool(name="dram", bufs=1, space="DRAM"))
        pers = top.enter_context(tc.tile_pool(name="pers", bufs=1))
        attn = top.enter_context(tc.tile_pool(name="attn", bufs=1))

        HB = D // 2
        kb0 = dram.tile([HB, ROWS], BF16)               # kT bounce, jt 0-3
        kb1 = dram.tile([HB, ROWS], BF16)               # kT bounce, jt 4-7
        vb = dram.tile([ROWS, D], F8)                   # own V rows, fp8
        KTg0 = dram.tile([GROUP * HB, ROWS], BF16)      # gathered kT, jt 0-3
        KTg1 = dram.tile([GROUP * HB, ROWS], BF16)      # gathered kT, jt 4-7
        Vg = dram.tile([GROUP * ROWS, D], F8)           # gathered V, fp8
        dden = dram.tile([1, H * ROWS], F32)            # denominator hop

        ones_w = pers.tile([P, P], BF16)
        nc.vector.memset(ones_w[:], 1.0)
        bqk_sb = pers.tile([P, 2 * KT], F32)
        nc.sync.dma_start(bqk_sb[:], bqk[:])
        sel_sb = pers.tile([2, 2 * HD], BF16)
        nc.sync.dma_start(sel_sb[:], sel[:])
        bvr_sb = pers.tile([1, D], BF16)
        nc.sync.dma_start(bvr_sb[:], bvr[:])
        bor_sb = pers.tile([1, D], BF16)
        nc.sync.dma_start(bor_sb[:], bor[:])
        bvb_sb = pers.tile([P, D], BF16)
        nc.gpsimd.partition_broadcast(bvb_sb[:], bvr_sb[:])
        bob_sb = pers.tile([P, D], BF16)
        nc.gpsimd.partition_broadcast(bob_sb[:], bor_sb[:])
        # per-head q^T slots: head 2j on partitions 0-63, head 2j+1 on
        # 64-127; no zero padding needed - score matmuls are row-tiled
        qT_sb = pers.tile([P, H * ROWS], BF16)

        kt_sb = attn.tile([P, 4 * KT * ROWS], BF16)     # gathered K^T
        v_aug = attn.tile([P, KEYT * H * HA], F8)       # [V_h | 1] slots
        v4 = v_aug[:].rearrange("p (t h a) -> p t h a", h=H, a=HA)
        nc.gpsimd.memset(v4[:, :, :, HD:HA], 1.0)       # ones column only
        # pair-packed normalized ctx^T: head 2j on partitions 0-63 of pair
        # slot j, head 2j+1 on partitions 64-127 (odd heads arrive via a
        # cross-partition SBUF DMA from ctx_odd)
        ctx_pair = attn.tile([P, (H // 2) * ROWS], BF16)
        ctx_odd = attn.tile([HD, (H // 2) * ROWS], BF16)
        wo_sb = attn.tile([P, KT * D], BF16)

        # warmup burst: lift the HAM clock gate to 8/8 while the first
        # input DMAs are still in flight
        with ExitStack() as warm:
            ps_w = warm.enter_context(
                tc.tile_pool(name="ps_w", bufs=1, space="PSUM"))
            psw = ps_w.tile([P, P], F32, name="psw")
            for _ in range(40):
                nc.tensor.matmul(psw[:], ones_w[:], ones_w[:],
                                 start=True, stop=True)

        with ExitStack() as proj:
            wpool = proj.enter_context(tc.tile_pool(name="wpool", bufs=1))
            epool = proj.enter_context(tc.tile_pool(name="epool", bufs=3))
            ps_proj = proj.enter_context(
                tc.tile_pool(name="ps_proj", bufs=3, space="PSUM"))

            wk_sb = wpool.tile([P, KT * D], BF16)
            hT_sb = wpool.tile([P, KT * ROWS], BF16)
            wv_sb = wpool.tile([P, KT * D], BF16)
            wq_sb = wpool.tile([P, KT * D], BF16)
            # urgent input DMAs only: hT + wk feed the K projection
            for kk in range(0, KT, 2):
                nc.sync.dma_start(
                    hT_sb[:, kk * ROWS:(kk + 2) * ROWS]
                    .rearrange("p (k f) -> p k f", f=ROWS),
                    hT[kk * P:(kk + 2) * P, :]
                    .rearrange("(k p) f -> p k f", p=P))
                nc.sync.dma_start(
                    wk_sb[:, kk * D:(kk + 2) * D]
                    .rearrange("p (k f) -> p k f", f=D),
                    wk[kk * P:(kk + 2) * P, :]
                    .rearrange("(k p) f -> p k f", p=P))

            def wload(dst, src, kk, n_k, eng):
                eng.dma_start(
                    dst[:, kk * D:(kk + n_k) * D]
                    .rearrange("p (k f) -> p k f", f=D),
                    src[kk * P:(kk + n_k) * P, :]
                    .rearrange("(k p) f -> p k f", p=P))

            # k^T projection (bias bk folded into DVE eviction)
            def kproj(ms, kbt):
                for m in ms:
                    ps = ps_proj.tile([P, ROWS], F32, name="ps")
                    for k in range(KT):
                        nc.tensor.matmul(
                            ps[:],
                            wk_sb[:, k * D + m * P: k * D + (m + 1) * P],
                            hT_sb[:, k * ROWS:(k + 1) * ROWS],
                            start=(k == 0), stop=(k == KT - 1))
                    ev = epool.tile([P, ROWS], BF16, name="ev")
                    nc.vector.tensor_scalar(
                        ev[:], ps[:], bqk_sb[:, KT + m: KT + m + 1], None,
                        ALU.add)
                    nc.sync.dma_start(
                        kbt[(m % 4) * P:(m % 4 + 1) * P, :], ev[:])

            kproj(range(0, 4), kb0)
            nc.gpsimd.collective_compute(
                "AllGather", mybir.AluOpType.bypass,
                replica_groups=[[0, 1, 2, 3], [4, 5, 6, 7]],
                ins=[kb0.opt()], outs=[KTg0.opt()])
            # wv/wq/wo loads fire from the GPSIMD stream after the AG_K0
            # trigger (which waits on the k^T evictions), so they cannot
            # race the hT/wk loads for HBM bandwidth
            for kk in (0, 4):
                wload(wv_sb, wv, kk, 4, nc.gpsimd)
            for kk in (0, 4):
                wload(wq_sb, wq, kk, 4, nc.gpsimd)
            for kk in (0, 4):
                wload(wo_sb, wo, kk, 4, nc.gpsimd)

            # V projection for own rows only -> fp8 bounce
            for mk in range(ROWS // P):
                for n in (0, 1):
                    ps = ps_proj.tile([P, 512], F32, name="ps")
                    for k in range(KT):
                        nc.tensor.matmul(
                            ps[:],
                            hT_sb[:, k * ROWS + mk * P: k * ROWS + (mk + 1) * P],
                            wv_sb[:, k * D + n * 512: k * D + (n + 1) * 512],
                            start=(k == 0), stop=(k == KT - 1))
                    evv = epool.tile([P, 512], F8, name="evv")
                    nc.vector.tensor_add(
                        evv[:], ps[:], bvb_sb[:, n * 512:(n + 1) * 512])
                    nc.sync.dma_start(
                        vb[mk * P:(mk + 1) * P, n * 512:(n + 1) * 512],
                        evv[:])
            nc.gpsimd.collective_compute(
                "AllGather", mybir.AluOpType.bypass,
                replica_groups=[[0, 1, 2, 3], [4, 5, 6, 7]],
                ins=[vb.opt()], outs=[Vg.opt()])

            # q^T projection -> per-head slots (DVE eviction)
            for m in range(KT):
                ps = ps_proj.tile([P, ROWS], F32, name="ps")
                for k in range(KT):
                    nc.tensor.matmul(
                        ps[:],
                        wq_sb[:, k * D + m * P: k * D + (m + 1) * P],
                        hT_sb[:, k * ROWS:(k + 1) * ROWS],
                        start=(k == 0), stop=(k == KT - 1))
                for hh in (0, 1):
                    h = 2 * m + hh
                    po = hh * HD
                    nc.vector.tensor_scalar(
                        qT_sb[po:po + HD, h * ROWS:(h + 1) * ROWS],
                        ps[po:po + HD, :],
                        bqk_sb[po:po + HD, m:m + 1], None,
                        ALU.add)

            kproj(range(4, 8), kb1)
            nc.gpsimd.collective_compute(
                "AllGather", mybir.AluOpType.bypass,
                replica_groups=[[0, 1, 2, 3], [4, 5, 6, 7]],
                ins=[kb1.opt()], outs=[KTg1.opt()])

            # gathered K^T halves into SBUF (jt 0-3 after AG1, 4-7 after AG3)
            for half, KTgh in ((0, KTg0), (1, KTg1)):
                for r in range(GROUP):
                    nc.sync.dma_start(
                        kt_sb[:, (r * KT + half * 4) * ROWS:
                              (r * KT + half * 4 + 4) * ROWS]
                        .rearrange("p (t f) -> p t f", f=ROWS),
                        KTgh[r * HB:(r + 1) * HB, :]
                        .rearrange("(t p) f -> p t f", p=P))
            # gathered V into the augmented [V_h | 1] slots
            for t in range(KEYT):
                nc.sync.dma_start(
                    v4[:, t, :, 0:HD],
                    Vg[t * P:(t + 1) * P, :]
                    .rearrange("p (h d) -> p h d", d=HD))

        with ExitStack() as att:
            probs = att.enter_context(tc.tile_pool(name="probs", bufs=24))
            ipool = att.enter_context(tc.tile_pool(name="ipool", bufs=2))
            norm = att.enter_context(tc.tile_pool(name="norm", bufs=3))
            late = att.enter_context(tc.tile_pool(name="late", bufs=1))
            # per-head unnormalized ctx stash (f32): V rows 0..63, denom 64
            stash = late.tile([HA, H * ROWS], F32)

            with ExitStack() as attp:
                ps_s = attp.enter_context(
                    tc.tile_pool(name="ps_s", bufs=2, space="PSUM"))
                ps_ctx = attp.enter_context(
                    tc.tile_pool(name="ps_ctx", bufs=2, space="PSUM"))
                ps_x = attp.enter_context(
                    tc.tile_pool(name="ps_x", bufs=1, space="PSUM"))

                U = KEYT // 2
                LAG_U = 10
                NP = H // 2
                pend = {}
                psc = {}

                def emit_scores(p, u):
                    tiles = []
                    for hh in (0, 1):
                        tiles.append(ps_s.tile([P, 2 * ROWS], F32,
                                               name="ps_sc"))
                    for half in (0, 1):
                        t = 2 * u + half
                        r, m = divmod(t, KEYT // GROUP)
                        base = (r * KT + p) * ROWS
                        for hh in (0, 1):
                            h = 2 * p + hh
                            po = hh * HD
                            nc.tensor.matmul(
                                tiles[hh][:, half * ROWS:(half + 1) * ROWS],
                                kt_sb[po:po + HD,
                                      base + m * P: base + (m + 1) * P],
                                qT_sb[po:po + HD, h * ROWS:(h + 1) * ROWS],
                                start=True, stop=True)
                    gidx = p * U + u
                    for hh in (0, 1):
                        pt = probs.tile([P, 2 * ROWS], F8, name="pt")
                        if hh == 0 or gidx % 5 == 4:
                            nc.scalar.activation(pt[:], tiles[hh][:],
                                                 AF.Exp, scale=0.125)
                        else:
                            # Schraudolph fast exp: int32 i = a*s + b
                            # approximates the f32 bit pattern of exp(s/8);
                            # DVE does the affine step, GPSIMD the bitcast
                            # + fp8 store
                            ti = ipool.tile([P, 2 * ROWS], dt.int32,
                                            name="ti")
                            nc.vector.tensor_scalar(
                                ti[:], tiles[hh][:],
                                12102203.16 / 8.0, 1064866805.0,
                                ALU.mult, ALU.add)
                            nc.gpsimd.tensor_copy(pt[:], ti[:].bitcast(F32))
                        pend[(2 * p + hh, u)] = pt

                def emit_ctx(p, u):
                    for hh in (0, 1):
                        h = 2 * p + hh
                        if u == 0:
                            psc[h] = ps_ctx.tile([HA, ROWS], F32, name="ps_c")
                        pt = pend.pop((h, u))
                        nc.tensor.matmul(
                            psc[h][:],
                            v4[:, 2 * u:2 * u + 2, h, :],
                            pt[:].rearrange("p (t f) -> p t f", f=ROWS),
                            start=(u == 0), stop=(u == U - 1),
                            perf_mode=DR)
                        if u == U - 1:
                            ps_c = psc.pop(h)
                            nc.vector.tensor_copy(
                                stash[:, h * ROWS:(h + 1) * ROWS], ps_c[:])

                def flush_pair(pq):
                    # denominators -> DRAM hop -> partitions 0-1 -> one
                    # fast reciprocal -> bf16 -> selector matmul broadcast
                    # -> DVE normalize multiply
                    h0 = 2 * pq
                    nc.sync.dma_start(
                        dden[0:1, h0 * ROWS:(h0 + 2) * ROWS],
                        stash[HD:HD + 1, h0 * ROWS:(h0 + 2) * ROWS])
                    rn = norm.tile([2, ROWS], F32, name="rn")
                    nc.sync.dma_start(
                        rn[:],
                        dden[0:1, h0 * ROWS:(h0 + 2) * ROWS]
                        .rearrange("p (h f) -> (p h) f", f=ROWS))
                    rr = norm.tile([2, ROWS], F32, name="rr")
                    nc.vector.reciprocal_approx_fast(rr[:], rn[:])
                    rb = norm.tile([2, ROWS], BF16, name="rb")
                    nc.vector.tensor_copy(rb[:], rr[:])
                    for hh in (0, 1):
                        h = h0 + hh
                        psb = ps_x.tile([P, ROWS], F32, name="psb")
                        nc.tensor.matmul(
                            psb[0:HD, :],
                            sel_sb[:, hh * HD:(hh + 1) * HD],
                            rb[:], start=True, stop=True)
                        if hh == 0:
                            dst = ctx_pair[0:HD, pq * ROWS:(pq + 1) * ROWS]
                        else:
                            dst = ctx_odd[:, pq * ROWS:(pq + 1) * ROWS]
                        nc.vector.tensor_mul(
                            dst, stash[0:HD, h * ROWS:(h + 1) * ROWS],
                            psb[0:HD, :])
                        if hh == 1:
                            nc.sync.dma_start(
                                ctx_pair[HD:P, pq * ROWS:(pq + 1) * ROWS],
                                ctx_odd[:, pq * ROWS:(pq + 1) * ROWS])

                for G in range(NP * U + LAG_U):
                    if LAG_U <= G:
                        pc, uc = divmod(G - LAG_U, U)
                        emit_ctx(pc, uc)
                        if uc == U - 1:
                            flush_pair(pc)
                    if G < NP * U:
                        emit_scores(G // U, G % U)

            with ExitStack() as outp_s:
                ps_o = outp_s.enter_context(
                    tc.tile_pool(name="ps_o", bufs=2, space="PSUM"))
                opool = outp_s.enter_context(
                    tc.tile_pool(name="opool", bufs=2))
                for m in range(ROWS // P):
                    for n in range(2):
                        ps = ps_o.tile([P, 512], F32, name="ps_out")
                        for j in range(H // 2):
                            nc.tensor.matmul(
                                ps[:],
                                ctx_pair[:, j * ROWS + m * P:
                                         j * ROWS + (m + 1) * P],
                                wo_sb[:, j * D + n * 512: j * D + (n + 1) * 512],
                                start=(j == 0), stop=(j == H // 2 - 1))
                        ot = opool.tile([P, 512], F32, name="ot")
                        nc.vector.tensor_add(
                            ot[:], ps[:], bob_sb[:, n * 512:(n + 1) * 512])
                        nc.sync.dma_start(
                            out[m * P:(m + 1) * P, n * 512:(n + 1) * 512],
                            ot[:])

    nc.compile()
    return nc


def _prep_inputs(hidden_states, Wq, bq, Wk, bk, Wv, bv, Wo, bo):
    hs = np.asarray(hidden_states, np.float32).reshape(B * S, D)
    wq = np.asarray(Wq, np.float32).astype(bf16)
    wk = np.asarray(Wk, np.float32).astype(bf16)
    wv = np.asarray(Wv, np.float32).astype(bf16)
    wo = np.asarray(Wo, np.float32).astype(bf16)
    bvr = np.asarray(bv, np.float32).reshape(1, D).astype(bf16)
    bor = np.asarray(bo, np.float32).reshape(1, D).astype(bf16)
    sel = np.zeros((2, 2 * HD), np.float32)
    sel[0, 0:HD] = 1.0
    sel[1, HD:2 * HD] = 1.0
    sel = sel.astype(bf16)
    bqk = np.ascontiguousarray(np.concatenate(
        [np.asarray(bq, np.float32).reshape(KT, P).T,
         np.asarray(bk, np.float32).reshape(KT, P).T], 1).astype(np.float32))
    in_maps = []
    for c in range(N_CORES):
        hT = np.ascontiguousarray(
            hs[c * ROWS:(c + 1) * ROWS].T).astype(bf16)
        in_maps.append({"hT": hT, "wq": wq, "wk": wk,
                        "wv": wv, "wo": wo, "bvr": bvr, "bor": bor,
                        "bqk": bqk, "sel": sel})
    return in_maps


def _run(inputs, trace=False):
    from concourse import bass_utils
    if "nc" not in _CACHE:
        _CACHE["nc"] = _build_graph()
    nc = _CACHE["nc"]
    in_maps = _prep_inputs(**inputs)
    res = bass_utils.run_bass_kernel_spmd(
        nc, in_maps, core_ids=list(range(N_CORES)), trace=trace)
    full = np.concatenate([res.results[c]["out"] for c in range(N_CORES)],
                          axis=0).reshape(B, S, D).astype(np.float32)
    return full, res


def kernel(**inputs) -> np.ndarray:
    full, _ = _run(inputs, trace=False)
    return full


# revision 7
# speedup vs baseline: 1.2574x; 1.2574x over previous
"""Distributed multi-head attention kernel for 8 TRN2 NeuronCores.

Problem: hidden[2,2048,1024] -> QKV proj (16 heads, hd=64) -> softmax
attention -> out proj. f32 I/O, bf16 tensor-engine compute; fp8e4
probs/V for a DoubleRow ctx contraction.

Sharding: sequence-parallel. Flattened rows [4096, 1024] split into 8
chunks of 512 rows; cores 0-3 own batch 0, cores 4-7 batch 1. Each core
projects K^T and V for its own 512 rows and AllGathers both within its
4-core batch group (K^T split in two bf16 halves, V as fp8), hidden
under the V/Q projections. Q^T is local. Each core then runs full
16-head attention + output projection for its 512 rows; outputs are
disjoint row blocks concatenated on the host.

PE budget: score matmuls are row-tiled - head 2p contracts on array
rows 0-63, head 2p+1 on rows 64-127 (tile_position from the APs'
base partitions), so the two 64-dim contractions run concurrently and
score throughput doubles vs zero-padded 128-dim matmuls. ctx uses
fp8 DoubleRow over two key tiles per matmul. A warmup matmul burst at
t=0 lifts the HAM clock gate before the first projection.

Engine budget: softmax exp is split three ways - ACT does head-even
tiles (plus every 5th head-odd), the rest run as Schraudolph fast-exp
with the int32 tensor_scalar on the DVE and the bitcast fp8 copy on
the otherwise-idle GPSIMD. Denominators stash in f32, hop through
DRAM, and invert with one reciprocal_approx_fast per head pair; the
per-pair flush interleaves with the ongoing score/ctx stream.
"""

import numpy as np
import ml_dtypes

B, S, D, H, HD = 2, 2048, 1024, 16, 64
N_CORES = 8
ROWS = (B * S) // N_CORES          # 512 query rows per core
GROUP = 4                          # cores per batch group
P = 128
KT = D // P                        # 8 contraction tiles over hidden dim
KEYT = S // P                      # 16 key tiles per batch
HA = HD + 1                        # head slot width in v_aug

_CACHE: dict = {}

bf16 = ml_dtypes.bfloat16
f8 = ml_dtypes.float8_e4m3


def _build_graph():
    import concourse.mybir as mybir
    import concourse.tile as tile
    from concourse import bacc
    from contextlib import ExitStack

    dt = mybir.dt
    F32, BF16, F8 = dt.float32, dt.bfloat16, dt.float8e4
    AF = mybir.ActivationFunctionType
    ALU = mybir.AluOpType
    DR = mybir.MatmulPerfMode.DoubleRow

    nc = bacc.Bacc("TRN2", target_bir_lowering=False, debug=False,
                   enable_asserts=False, num_devices=N_CORES)

    hT = nc.dram_tensor("hT", [D, ROWS], BF16, kind="ExternalInput").ap()
    wq = nc.dram_tensor("wq", [D, D], BF16, kind="ExternalInput").ap()
    wk = nc.dram_tensor("wk", [D, D], BF16, kind="ExternalInput").ap()
    wv = nc.dram_tensor("wv", [D, D], BF16, kind="ExternalInput").ap()
    wo = nc.dram_tensor("wo", [D, D], BF16, kind="ExternalInput").ap()
    bvr = nc.dram_tensor("bvr", [1, D], BF16, kind="ExternalInput").ap()
    bor = nc.dram_tensor("bor", [1, D], BF16, kind="ExternalInput").ap()
    bqk = nc.dram_tensor("bqk", [P, 2 * KT], F32, kind="ExternalInput").ap()
    sel = nc.dram_tensor("sel", [2, 2 * HD], BF16, kind="ExternalInput").ap()
    out = nc.dram_tensor("out", [ROWS, D], F32, kind="ExternalOutput").ap()

    with tile.TileContext(nc) as tc, ExitStack() as top:
        dram = top.enter_context(tc.tile_pool(name="dram", bufs=1, space="DRAM"))
        pers = top.enter_context(tc.tile_pool(name="pers", bufs=1))
        attn = top.enter_context(tc.tile_pool(name="attn", bufs=1))

        HB = D // 2
        kb0 = dram.tile([HB, ROWS], BF16)               # kT bounce, jt 0-3
        kb1 = dram.tile([HB, ROWS], BF16)               # kT bounce, jt 4-7
        vb = dram.tile([ROWS, D], F8)                   # own V rows, fp8
        KTg0 = dram.tile([GROUP * HB, ROWS], BF16)      # gathered kT, jt 0-3
        KTg1 = dram.tile([GROUP * HB, ROWS], BF16)      # gathered kT, jt 4-7
        Vg = dram.tile([GROUP * ROWS, D], F8)           # gathered V, fp8
        dden = dram.tile([1, H * ROWS], F32)            # denominator hop

        ones_w = pers.tile([P, P], BF16)
        nc.vector.memset(ones_w[:], 1.0)
        bqk_sb = pers.tile([P, 2 * KT], F32)
        nc.sync.dma_start(bqk_sb[:], bqk[:])
        sel_sb = pers.tile([2, 2 * HD], BF16)
        nc.sync.dma_start(sel_sb[:], sel[:])
        bvr_sb = pers.tile([1, D], BF16)
        nc.sync.dma_start(bvr_sb[:], bvr[:])
        bor_sb = pers.tile([1, D], BF16)
        nc.sync.dma_start(bor_sb[:], bor[:])
        bvb_sb = pers.tile([P, D], BF16)
        nc.gpsimd.partition_broadcast(bvb_sb[:], bvr_sb[:])
        bob_sb = pers.tile([P, D], BF16)
        nc.gpsimd.partition_broadcast(bob_sb[:], bor_sb[:])
        # per-head q^T slots: head 2j on partitions 0-63, head 2j+1 on
        # 64-127; no zero padding needed - score matmuls are row-tiled
        qT_sb = pers.tile([P, H * ROWS], BF16)

        kt_sb = attn.tile([P, 4 * KT * ROWS], BF16)     # gathered K^T
        v_aug = attn.tile([P, KEYT * H * HA], F8)       # [V_h | 1] slots
        v4 = v_aug[:].rearrange("p (t h a) -> p t h a", h=H, a=HA)
        nc.gpsimd.memset(v4[:, :, :, HD:HA], 1.0)       # ones column only
        # pair-packed normalized ctx^T: head 2j on partitions 0-63 of pair
        # slot j, head 2j+1 on partitions 64-127 (odd heads arrive via a
        # cross-partition SBUF DMA from ctx_odd)
        ctx_pair = attn.tile([P, (H // 2) * ROWS], BF16)
        ctx_odd = attn.tile([HD, (H // 2) * ROWS], BF16)
        wo_sb = attn.tile([P, KT * D], BF16)

        # warmup burst: lift the HAM clock gate to 8/8 while the first
        # input DMAs are still in flight
        with ExitStack() as warm:
            ps_w = warm.enter_context(
                tc.tile_pool(name="ps_w", bufs=1, space="PSUM"))
            psw = ps_w.tile([P, P], F32, name="psw")
            for _ in range(40):
                nc.tensor.matmul(psw[:], ones_w[:], ones_w[:],
                                 start=True, stop=True)

        with ExitStack() as proj:
            wpool = proj.enter_context(tc.tile_pool(name="wpool", bufs=1))
            epool = proj.enter_context(tc.tile_pool(name="epool", bufs=3))
            ps_proj = proj.enter_context(
                tc.tile_pool(name="ps_proj", bufs=3, space="PSUM"))

            wk_sb = wpool.tile([P, KT * D], BF16)
            hT_sb = wpool.tile([P, KT * ROWS], BF16)
            wv_sb = wpool.tile([P, KT * D], BF16)
            wq_sb = wpool.tile([P, KT * D], BF16)
            # urgent input DMAs only: hT + wk feed the K projection
            for kk in range(0, KT, 2):
                nc.sync.dma_start(
                    hT_sb[:, kk * ROWS:(kk + 2) * ROWS]
                    .rearrange("p (k f) -> p k f", f=ROWS),
                    hT[kk * P:(kk + 2) * P, :]
                    .rearrange("(k p) f -> p k f", p=P))
                nc.sync.dma_start(
                    wk_sb[:, kk * D:(kk + 2) * D]
                    .rearrange("p (k f) -> p k f", f=D),
                    wk[kk * P:(kk + 2) * P, :]
                    .rearrange("(k p) f -> p k f", p=P))

            def wload(dst, src, kk, n_k, eng):
                eng.dma_start(
                    dst[:, kk * D:(kk + n_k) * D]
                    .rearrange("p (k f) -> p k f", f=D),
                    src[kk * P:(kk + n_k) * P, :]
                    .rearrange("(k p) f -> p k f", p=P))

            # k^T projection (bias bk folded into DVE eviction)
            def kproj(ms, kbt):
                for m in ms:
                    ps = ps_proj.tile([P, ROWS], F32, name="ps")
                    for k in range(KT):
                        nc.tensor.matmul(
                            ps[:],
                            wk_sb[:, k * D + m * P: k * D + (m + 1) * P],
                            hT_sb[:, k * ROWS:(k + 1) * ROWS],
                            start=(k == 0), stop=(k == KT - 1))
                    ev = epool.tile([P, ROWS], BF16, name="ev")
                    nc.vector.tensor_scalar(
                        ev[:], ps[:], bqk_sb[:, KT + m: KT + m + 1], None,
                        ALU.add)
                    nc.sync.dma_start(
                        kbt[(m % 4) * P:(m % 4 + 1) * P, :], ev[:])

            kproj(range(0, 4), kb0)
            nc.gpsimd.collective_compute(
                "AllGather", mybir.AluOpType.bypass,
                replica_groups=[[0, 1, 2, 3], [4, 5, 6, 7]],
                ins=[kb0.opt()], outs=[KTg0.opt()])
            # wv/wq/wo loads fire from the GPSIMD stream after the AG_K0
            # trigger (which waits on the k^T evictions), so they cannot
            # race the hT/wk loads for HBM bandwidth
            for kk in (0, 4):
                wload(wv_sb, wv, kk, 4, nc.gpsimd)
            for kk in (0, 4):
                wload(wq_sb, wq, kk, 4, nc.gpsimd)
            for kk in (0, 4):
                wload(wo_sb, wo, kk, 4, nc.gpsimd)

            # V projection for own rows only -> fp8 bounce
            for mk in range(ROWS // P):
                for n in (0, 1):
                    ps = ps_proj.tile([P, 512], F32, name="ps")
                    for k in range(KT):
                        nc.tensor.matmul(
                            ps[:],
                            hT_sb[:, k * ROWS + mk * P: k * ROWS + (mk + 1) * P],
                            wv_sb[:, k * D + n * 512: k * D + (n + 1) * 512],
                            start=(k == 0), stop=(k == KT - 1))
                    evv = epool.tile([P, 512], F8, name="evv")
                    nc.vector.tensor_add(
                        evv[:], ps[:], bvb_sb[:, n * 512:(n + 1) * 512])
                    nc.sync.dma_start(
                        vb[mk * P:(mk + 1) * P, n * 512:(n + 1) * 512],
                        evv[:])
            nc.gpsimd.collective_compute(
                "AllGather", mybir.AluOpType.bypass,
                replica_groups=[[0, 1, 2, 3], [4, 5, 6, 7]],
                ins=[vb.opt()], outs=[Vg.opt()])

            # q^T projection -> per-head slots (DVE eviction)
            for m in range(KT):
                ps = ps_proj.tile([P, ROWS], F32, name="ps")
                for k in range(KT):
                    nc.tensor.matmul(
                        ps[:],
                        wq_sb[:, k * D + m * P: k * D + (m + 1) * P],
                        hT_sb[:, k * ROWS:(k + 1) * ROWS],
                        start=(k == 0), stop=(k == KT - 1))
                for hh in (0, 1):
                    h = 2 * m + hh
                    po = hh * HD
                    nc.vector.tensor_scalar(
                        qT_sb[po:po + HD, h * ROWS:(h + 1) * ROWS],
                        ps[po:po + HD, :],
                        bqk_sb[po:po + HD, m:m + 1], None,
                        ALU.add)

            kproj(range(4, 8), kb1)
            nc.gpsimd.collective_compute(
                "AllGather", mybir.AluOpType.bypass,
                replica_groups=[[0, 1, 2, 3], [4, 5, 6, 7]],
                ins=[kb1.opt()], outs=[KTg1.opt()])

            # gathered K^T halves into SBUF (jt 0-3 after AG1, 4-7 after AG3)
            for half, KTgh in ((0, KTg0), (1, KTg1)):
                for r in range(GROUP):
                    nc.sync.dma_start(
                        kt_sb[:, (r * KT + half * 4) * ROWS:
                              (r * KT + half * 4 + 4) * ROWS]
                        .rearrange("p (t f) -> p t f", f=ROWS),
                        KTgh[r * HB:(r + 1) * HB, :]
                        .rearrange("(t p) f -> p t f", p=P))
            # gathered V into the augmented [V_h | 1] slots
            for t in range(KEYT):
                nc.sync.dma_start(
                    v4[:, t, :, 0:HD],
                    Vg[t * P:(t + 1) * P, :]
                    .rearrange("p (h d) -> p h d", d=HD))

        with ExitStack() as att:
            probs = att.enter_context(tc.tile_pool(name="probs", bufs=24))
            norm = att.enter_context(tc.tile_pool(name="norm", bufs=3))
            late = att.enter_context(tc.tile_pool(name="late", bufs=1))
            # per-head unnormalized ctx stash (f32): V rows 0..63, denom 64
            stash = late.tile([HA, H * ROWS], F32)

            with ExitStack() as attp:
                ps_s = attp.enter_context(
                    tc.tile_pool(name="ps_s", bufs=2, space="PSUM"))
                ps_ctx = attp.enter_context(
                    tc.tile_pool(name="ps_ctx", bufs=2, space="PSUM"))
                ps_x = attp.enter_context(
                    tc.tile_pool(name="ps_x", bufs=1, space="PSUM"))

                U = KEYT // 2
                LAG_U = 10
                NP = H // 2
                pend = {}
                psc = {}

                def emit_scores(p, u):
                    tiles = []
                    for hh in (0, 1):
                        tiles.append(ps_s.tile([P, 2 * ROWS], F32,
                                               name="ps_sc"))
                    for half in (0, 1):
                        t = 2 * u + half
                        r, m = divmod(t, KEYT // GROUP)
                        base = (r * KT + p) * ROWS
                        for hh in (0, 1):
                            h = 2 * p + hh
                            po = hh * HD
                            nc.tensor.matmul(
                                tiles[hh][:, half * ROWS:(half + 1) * ROWS],
                                kt_sb[po:po + HD,
                                      base + m * P: base + (m + 1) * P],
                                qT_sb[po:po + HD, h * ROWS:(h + 1) * ROWS],
                                start=True, stop=True)
                    gidx = p * U + u
                    for hh in (0, 1):
                        pt = probs.tile([P, 2 * ROWS], F8, name="pt")
                        if hh == 0 or gidx % 4 == 3:
                            nc.scalar.activation(pt[:], tiles[hh][:],
                                                 AF.Exp, scale=0.125)
                        else:
                            # Schraudolph fast exp straight to fp8: the
                            # int8 value 1.4427*s + 55.65 IS the fp8e4m3
                            # bit pattern of exp(s/8) (8*(log2+7) with the
                            # mantissa-curvature shift) - one DVE op
                            nc.vector.tensor_scalar(
                                pt[:].bitcast(dt.int8), tiles[hh][:],
                                1.442695, 55.65,
                                ALU.mult, ALU.add)
                        pend[(2 * p + hh, u)] = pt

                def emit_ctx(p, u):
                    for hh in (0, 1):
                        h = 2 * p + hh
                        if u == 0:
                            psc[h] = ps_ctx.tile([HA, ROWS], F32, name="ps_c")
                        pt = pend.pop((h, u))
                        nc.tensor.matmul(
                            psc[h][:],
                            v4[:, 2 * u:2 * u + 2, h, :],
                            pt[:].rearrange("p (t f) -> p t f", f=ROWS),
                            start=(u == 0), stop=(u == U - 1),
                            perf_mode=DR)
                        if u == U - 1:
                            ps_c = psc.pop(h)
                            nc.vector.tensor_copy(
                                stash[:, h * ROWS:(h + 1) * ROWS], ps_c[:])

                def flush_pair(pq):
                    # denominators -> DRAM hop -> partitions 0-1 -> one
                    # fast reciprocal -> bf16 -> selector matmul broadcast
                    # -> DVE normalize multiply
                    h0 = 2 * pq
                    nc.sync.dma_start(
                        dden[0:1, h0 * ROWS:(h0 + 2) * ROWS],
                        stash[HD:HD + 1, h0 * ROWS:(h0 + 2) * ROWS])
                    rn = norm.tile([2, ROWS], F32, name="rn")
                    nc.sync.dma_start(
                        rn[:],
                        dden[0:1, h0 * ROWS:(h0 + 2) * ROWS]
                        .rearrange("p (h f) -> (p h) f", f=ROWS))
                    rr = norm.tile([2, ROWS], F32, name="rr")
                    nc.vector.reciprocal_approx_fast(rr[:], rn[:])
                    rb = norm.tile([2, ROWS], BF16, name="rb")
                    nc.vector.tensor_copy(rb[:], rr[:])
                    for hh in (0, 1):
                        h = h0 + hh
                        psb = ps_x.tile([P, ROWS], F32, name="psb")
                        nc.tensor.matmul(
                            psb[0:HD, :],
                            sel_sb[:, hh * HD:(hh + 1) * HD],
                            rb[:], start=True, stop=True)
                        if hh == 0:
                            dst = ctx_pair[0:HD, pq * ROWS:(pq + 1) * ROWS]
                        else:
                            dst = ctx_odd[:, pq * ROWS:(pq + 1) * ROWS]
                        nc.vector.tensor_mul(
                            dst, stash[0:HD, h * ROWS:(h + 1) * ROWS],
                            psb[0:HD, :])
                        if hh == 1:
                            nc.sync.dma_start(
                                ctx_pair[HD:P, pq * ROWS:(pq + 1) * ROWS],
                                ctx_odd[:, pq * ROWS:(pq + 1) * ROWS])

                for G in range(NP * U + LAG_U):
                    if LAG_U <= G:
                        pc, uc = divmod(G - LAG_U, U)
                        emit_ctx(pc, uc)
                        if uc == U - 1:
                            flush_pair(pc)
                    if G < NP * U:
                        emit_scores(G // U, G % U)

            with ExitStack() as outp_s:
                ps_o = outp_s.enter_context(
                    tc.tile_pool(name="ps_o", bufs=2, space="PSUM"))
                opool = outp_s.enter_context(
                    tc.tile_pool(name="opool", bufs=2))
                for m in range(ROWS // P):
                    for n in range(2):
                        ps = ps_o.tile([P, 512], F32, name="ps_out")
                        for j in range(H // 2):
                            nc.tensor.matmul(
                                ps[:],
                                ctx_pair[:, j * ROWS + m * P:
                                         j * ROWS + (m + 1) * P],
                                wo_sb[:, j * D + n * 512: j * D + (n + 1) * 512],
                                start=(j == 0), stop=(j == H // 2 - 1))
                        ot = opool.tile([P, 512], F32, name="ot")
                        nc.vector.tensor_add(
                            ot[:], ps[:], bob_sb[:, n * 512:(n + 1) * 512])
                        nc.sync.dma_start(
                            out[m * P:(m + 1) * P, n * 512:(n + 1) * 512],
                            ot[:])

    nc.compile()
    return nc


def _prep_inputs(hidden_states, Wq, bq, Wk, bk, Wv, bv, Wo, bo):
    hs = np.asarray(hidden_states, np.float32).reshape(B * S, D)
    wq = np.asarray(Wq, np.float32).astype(bf16)
    wk = np.asarray(Wk, np.float32).astype(bf16)
    wv = np.asarray(Wv, np.float32).astype(bf16)
    wo = np.asarray(Wo, np.float32).astype(bf16)
    bvr = np.asarray(bv, np.float32).reshape(1, D).astype(bf16)
    bor = np.asarray(bo, np.float32).reshape(1, D).astype(bf16)
    sel = np.zeros((2, 2 * HD), np.float32)
    sel[0, 0:HD] = 1.0
    sel[1, HD:2 * HD] = 1.0
    sel = sel.astype(bf16)
    bqk = np.ascontiguousarray(np.concatenate(
        [np.asarray(bq, np.float32).reshape(KT, P).T,
         np.asarray(bk, np.float32).reshape(KT, P).T], 1).astype(np.float32))
    in_maps = []
    for c in range(N_CORES):
        hT = np.ascontiguousarray(
            hs[c * ROWS:(c + 1) * ROWS].T).astype(bf16)
        in_maps.append({"hT": hT, "wq": wq, "wk": wk,
                        "wv": wv, "wo": wo, "bvr": bvr, "bor": bor,
                        "bqk": bqk, "sel": sel})
    return in_maps


def _run(inputs, trace=False):
    from concourse import bass_utils
    if "nc" not in _CACHE:
        _CACHE["nc"] = _build_graph()
    nc = _CACHE["nc"]
    in_maps = _prep_inputs(**inputs)
    res = bass_utils.run_bass_kernel_spmd(
        nc, in_maps, core_ids=list(range(N_CORES)), trace=trace)
    full = np.concatenate([res.results[c]["out"] for c in range(N_CORES)],
                          axis=0).reshape(B, S, D).astype(np.float32)
    return full, res


def kernel(**inputs) -> np.ndarray:
    full, _ = _run(inputs, trace=False)
    return full
